# revision 1
# baseline (speedup 1.0000x reference)
"""Distributed masked-softmax attention for Trainium2 (8 NeuronCores).

Problem: B=2, H=16, S=2048, D=64 attention returning BOTH the [B,H,S,S]
attention-probability matrix and the [B,H,S,D] weighted values.

Sharding: the 32 (batch, head) pairs are split 4-per-core across 8 cores;
each core computes its heads fully independently (no collectives).

Per-core algorithm (per head):
  - Host passes Q^T (pre-scaled by 1/sqrt(D), with an appended ones-row) and
    K^T (with an appended mask-bias row: -10000 where masked) as float32r
    [65, S]; the single f32r matmul kT_chunk.T @ qT then yields
    scoresT[k, q] = q.k/8 + mask_bias[k] -- masking costs nothing.
  - exp on ScalarE evacuates PSUM -> SBUF as bf16 (no max-subtraction needed:
    scores are O(5) so exp cannot overflow; masked lanes underflow to 0
    exactly, matching the reference's exp(-1e30 - max) = 0).
  - PV matmul: V is host-augmented with a ones-column ([k, 65] bf16,
    stationary) so one accumulation chain produces both sum_valueT[d, q] and
    the softmax denominators (row 64).
  - sum_valueT chunks are PE-transposed back to [q, d]; the denominator
    column rides along, giving a per-partition 1/sum vector after
    nc.vector.reciprocal.
  - The attention output needs [q, k] layout in DRAM: each esc^T block is
    PE-transposed (bf16, 1 cycle/row) and the PSUM->SBUF evacuation fuses the
    1/sum scaling (per-partition scalar after the transpose).
"""
import sys

sys.path.insert(0, "/opt/trn_rl_repo")

import numpy as np
import ml_dtypes

import concourse.bacc as bacc
import concourse.tile as tile
import concourse.mybir as mybir
from concourse.masks import make_identity
from concourse.bass_utils import run_bass_kernel_spmd

F32 = mybir.dt.float32
F32R = mybir.dt.float32r
BF16 = mybir.dt.bfloat16
EXP = mybir.ActivationFunctionType.Exp

B, H, S, D = 2, 16, 2048, 64
N_CORES = 8
HPC = (B * H) // N_CORES  # heads per core = 4
KC = S // 128  # 16 k-chunks per head
QB = S // 128  # 16 q-blocks per head
DA = D + 1  # value augmented with ones column

# pool sizing knobs
ESC_BUFS = 20
ATT_BUFS = 3
SC_BUFS = 2
PV_BUFS = 2
TR_BUFS = 2


def build(repeat: int = 1):
    nc = bacc.Bacc("TRN2", target_bir_lowering=False, debug=False)
    qT_ext = nc.declare_dram_parameter("qT", [HPC, DA, S], F32R, isOutput=False)
    kT_ext = nc.declare_dram_parameter("kT", [HPC, DA, S], F32R, isOutput=False)
    vA_ext = nc.declare_dram_parameter("vA", [HPC, 128, KC, DA], BF16, isOutput=False)
    attn_ext = nc.declare_dram_parameter("attn", [HPC, S, S], F32, isOutput=True)
    sv_ext = nc.declare_dram_parameter("sv", [HPC, S, D], F32, isOutput=True)

    with tile.TileContext(nc) as tc:
        with (
            tc.tile_pool(name="consts", bufs=1) as consts,
            tc.tile_pool(name="io", bufs=2) as io,
            tc.tile_pool(name="esc", bufs=ESC_BUFS) as escp,
            tc.tile_pool(name="att", bufs=ATT_BUFS) as attp,
            tc.tile_pool(name="small", bufs=4) as small,
            tc.tile_pool(name="rcolp", bufs=2 * QB) as rcolp,
            tc.tile_pool(name="ps_sc", bufs=SC_BUFS, space="PSUM") as ps_sc,
            tc.tile_pool(name="ps_pv", bufs=PV_BUFS, space="PSUM") as ps_pv,
            tc.tile_pool(name="ps_tr", bufs=TR_BUFS, space="PSUM") as ps_tr,
        ):
            ident_bf = consts.tile([128, 128], BF16)
            make_identity(nc, ident_bf[:])
            ident_f = consts.tile([DA, DA], F32)
            make_identity(nc, ident_f[:])

            def head_body(h):
                qT_sb = io.tile([DA, S], F32R, tag="qT")
                nc.sync.dma_start(out=qT_sb[:], in_=qT_ext[h])
                kT_sb = io.tile([DA, S], F32R, tag="kT")
                nc.sync.dma_start(out=kT_sb[:], in_=kT_ext[h])
                vA_sb = io.tile([128, KC, DA], BF16, tag="vA")
                nc.sync.dma_start(out=vA_sb[:], in_=vA_ext[h])

                # --- scoresT = K_aug^T q-chunks, exp'd to bf16 SBUF ---
                esc = []
                for kc in range(KC):
                    e = escp.tile([128, S], BF16, tag="esc")
                    esc.append(e)
                    lhs = kT_sb[:, kc * 128 : (kc + 1) * 128]
                    for qh in range(2):
                        ps = ps_sc.tile([128, 1024], F32, tag="sc")
                        for i in range(2):
                            q0 = qh * 1024 + i * 512
                            nc.tensor.matmul(
                                ps[:, i * 512 : (i + 1) * 512],
                                lhs,
                                qT_sb[:, q0 : q0 + 512],
                                start=True,
                                stop=True,
                            )
                        nc.scalar.activation(
                            e[:, qh * 1024 : (qh + 1) * 1024], ps[:], EXP
                        )

                # --- PV + denominators; transpose sum chunks back to [q, d] ---
                rcols = []
                for qc in range(4):
                    pv = ps_pv.tile([DA, 512], F32, tag="pv")
                    for kc in range(KC):
                        nc.tensor.matmul(
                            pv[:],
                            vA_sb[:, kc, :],
                            esc[kc][:, qc * 512 : (qc + 1) * 512],
                            start=(kc == 0),
                            stop=(kc == KC - 1),
                        )
                    svt = small.tile([DA, 512], F32, tag="svt")
                    nc.vector.tensor_copy(svt[:], pv[:])
                    svo = small.tile([128, 4, D], F32, tag="svo")
                    for j in range(4):
                        pst = ps_tr.tile([128, DA], F32, tag="tr")
                        nc.tensor.transpose(
                            pst[:],
                            svt[:, j * 128 : (j + 1) * 128],
                            ident_f[:],
                        )
                        rcol = rcolp.tile([128, 1], F32, tag="rcol")
                        nc.vector.reciprocal(rcol[:], pst[:, D : D + 1])
                        rcols.append(rcol)
                        nc.vector.tensor_scalar_mul(svo[:, j, :], pst[:, :D], rcol[:])
                    sv_dst = sv_ext[h, qc * 512 : (qc + 1) * 512, :].rearrange(
                        "(j p) d -> p j d", p=128
                    )
                    nc.sync.dma_start(out=sv_dst, in_=svo[:])

                # --- attention output: transpose esc blocks, scale, store ---
                for qb in range(QB):
                    att = attp.tile([128, S], F32, tag="att")
                    for kg in range(4):
                        ptr = ps_tr.tile([128, 512], BF16, tag="tr")
                        for t in range(4):
                            kc = kg * 4 + t
                            nc.tensor.transpose(
                                ptr[:, t * 128 : (t + 1) * 128],
                                esc[kc][:, qb * 128 : (qb + 1) * 128],
                                ident_bf[:],
                            )
                        nc.any.tensor_scalar_mul(
                            att[:, kg * 512 : (kg + 1) * 512], ptr[:], rcols[qb][:]
                        )
                    nc.sync.dma_start(
                        out=attn_ext[h, qb * 128 : (qb + 1) * 128, :], in_=att[:]
                    )

            def full_body(_=None):
                for h in range(HPC):
                    head_body(h)

            if repeat == 1:
                full_body()
            else:
                with tc.For_i(0, repeat, 1):
                    full_body()

    nc.compile()
    return nc


_NC_CACHE = {}


def get_nc(repeat: int = 1):
    if repeat not in _NC_CACHE:
        _NC_CACHE[repeat] = build(repeat)
    return _NC_CACHE[repeat]


def prep_in_maps(query, key, value, mask):
    """Host-side shard + layout prep. Returns in_maps for cores 0..7."""
    q = np.asarray(query, dtype=np.float32).reshape(B * H, S, D)
    k = np.asarray(key, dtype=np.float32).reshape(B * H, S, D)
    v = np.asarray(value, dtype=np.float32).reshape(B * H, S, D)
    m = np.asarray(mask).reshape(B, S)

    qT = np.zeros((B * H, DA, S), dtype=np.float32)
    qT[:, :D, :] = q.transpose(0, 2, 1) * (1.0 / np.sqrt(D))
    qT[:, D, :] = 1.0

    kT = np.zeros((B * H, DA, S), dtype=np.float32)
    kT[:, :D, :] = k.transpose(0, 2, 1)
    mask_bias = np.where(m, np.float32(-10000.0), np.float32(0.0))  # [B, S]
    kT[:, D, :] = np.repeat(mask_bias, H, axis=0)

    vA = np.ones((B * H, S, DA), dtype=np.float32)
    vA[:, :, :D] = v
    # swizzle [S, DA] -> [128, KC, DA] so per-partition DMA runs are contiguous
    vA = vA.reshape(B * H, KC, 128, DA).transpose(0, 2, 1, 3)
    vA = vA.astype(ml_dtypes.bfloat16)

    in_maps = []
    for c in range(N_CORES):
        sl = slice(c * HPC, (c + 1) * HPC)
        in_maps.append({"qT": qT[sl], "kT": kT[sl], "vA": np.ascontiguousarray(vA[sl])})
    return in_maps


def kernel(query, key, value, mask):
    nc = get_nc(repeat=1)
    in_maps = prep_in_maps(query, key, value, mask)
    res = run_bass_kernel_spmd(nc, in_maps, core_ids=list(range(N_CORES))).results

    attn = np.concatenate([r["attn"] for r in res], axis=0).reshape(B, H, S, S)
    sv = np.concatenate([r["sv"] for r in res], axis=0).reshape(B, H, S, D)
    return attn, sv
